# revision 29
# baseline (speedup 1.0000x reference)
"""Trainium2 Bass kernel for nn_MultiHeadDuelingDQN (8-core SPMD), v2.

Model (B=256, STATE=26240, H=512, R=4000, N=64 heads, M=10):
    h  = relu(relu(x@W1+b1)@W2+b2)
    q_cache = h@Wvc+bvc + (h@Wac+bac) - mean_R(h@Wac+bac)
    q_assoc = per-head dueling over M (local means)
    q_rec   = S - mean_R(S),  S = sum_n (h@Wru[n]+bru[n])   [exact rewrite:
              rec_global has zero row-mean so the second mean is a no-op]

Design (v8; baseline v1 was ~344us, this is ~145us/core + launch skew):
  - Everything streams and computes in bf16 (halves HBM traffic, 4x matmul
    rate vs fp32); fp32 only in PSUM accumulation and final means.
  - All weights host-repacked into exact SBUF images [128, X]: every load is
    one contiguous-per-partition DMA; ZERO device transposes. Trunk computes
    h1T/h2T [h, b] directly (lhsT = natural W1/W2 k-chunks, rhs = host-
    transposed xT); heads consume h2T as lhsT. b1/b2 are per-partition ACT
    biases fused into relu+cast.
  - Wru head-sum: DVE pairwise TT-add tree over 16-head-block supertiles
    [128, 8000] bf16 (unit-stride bf16 TT hits the DVE 2x_1P mode;
    tensor_reduce does not). DVE does nothing else during the stream. The
    last supertile is split in half to shrink the post-stream DVE lag.
  - fc1 contraction-sharded; partials combined with ONE bf16 AllReduce.
    Measured on this fabric: AllReduce rides the CCE datapath and does NOT
    starve the model DMA queues, while AllToAll/ReduceScatter stall the Wru
    stream ~25-30us; AR output is the full sum so no AllGather is needed.
    fc2 + all heads run redundantly/locally (trivial vs the stream).
  - Full-R row-means (cache+rec) share ONE tiny tail AllGather; the rec
    finalize (qr) runs on DVE straight from PSUM while ACT does qc, and tail
    output DMAs go on the idle sync queue.
  - psS accumulation interleaved by k-chunk so only the last acc[3] matmul
    waits for the stream end.

kernel(**inputs) takes full unsharded fp32 inputs, returns full [256, 8640].
"""
import os
os.environ.setdefault("NEURON_RT_DBG_RDH_CC", "0")

import numpy as np
import ml_dtypes

import concourse.bass as bass
import concourse.mybir as mybir
import concourse.tile as tile
from concourse import bacc
from concourse import bass_utils
from concourse.bass import ts

NC = 8
B, H, STATE, R, NH, M = 256, 512, 26240, 4000, 64, 10
KPC_RAW = STATE // NC          # 3280
KCH = 26                       # k-chunks of 128 per core (padded)
KPC = KCH * 128                # 3328
RPC = R // NC                  # 500
HPC = NH // NC                 # 8 heads per core
AUG = HPC * (M + 1) + 1        # 89 = [8x(10 adv + 1 val)] + value_c
GRP = 16                       # heads per wru supertile
NGRP = NH // GRP               # 4 supertiles per k-chunk
F32 = mybir.dt.float32
BF16 = mybir.dt.bfloat16
RELU = mybir.ActivationFunctionType.Relu
COPY = mybir.ActivationFunctionType.Copy
IDENT = mybir.ActivationFunctionType.Identity
ADD = mybir.AluOpType.add
BF = ml_dtypes.bfloat16


def build_program(wru_bufs=6):
    nc = bacc.Bacc("TRN2", target_bir_lowering=False, debug=False, num_devices=NC)

    # ---- per-core I/O (all host-packed to exact SBUF images) ----
    xt = nc.dram_tensor("xt", [128, KCH * B], BF16, kind="ExternalInput").ap()
    w1 = nc.dram_tensor("w1", [128, KCH * H], BF16, kind="ExternalInput").ap()
    b1p = nc.dram_tensor("b1p", [128, 4], F32, kind="ExternalInput").ap()
    w2p = nc.dram_tensor("w2p", [128, 4 * H], BF16, kind="ExternalInput").ap()
    b2p = nc.dram_tensor("b2p", [128, 4], F32, kind="ExternalInput").ap()
    wacp = nc.dram_tensor("wacp", [128, 4 * RPC], BF16, kind="ExternalInput").ap()
    bacp = nc.dram_tensor("bacp", [1, RPC], BF16, kind="ExternalInput").ap()
    # supertiles: [kc, g, p, r*GRP+gi] = Wru[g*GRP+gi, kc*128+p, r0+r]
    wrup = nc.dram_tensor("wrup", [4, NGRP, 128, GRP * RPC], BF16,
                          kind="ExternalInput").ap()
    brup = nc.dram_tensor("brup", [NH, RPC], BF16, kind="ExternalInput").ap()
    augp = nc.dram_tensor("augp", [128, 4 * AUG], BF16, kind="ExternalInput").ap()
    augb = nc.dram_tensor("augb", [1, AUG], BF16, kind="ExternalInput").ap()

    out_cache = nc.dram_tensor("out_cache", [B, RPC], BF16, kind="ExternalOutput").ap()
    out_rec = nc.dram_tensor("out_rec", [B, RPC], BF16, kind="ExternalOutput").ap()
    out_assoc = nc.dram_tensor("out_assoc", [B, HPC * M], F32, kind="ExternalOutput").ap()

    with tile.TileContext(nc) as tc, \
         nc.allow_low_precision(reason="bf16 stream reduction; gate is 2e-2"):
        with (
            tc.tile_pool(name="cst", bufs=1) as cst,
            tc.tile_pool(name="sb", bufs=1) as sb,
            tc.tile_pool(name="wrupool", bufs=wru_bufs) as wrupool,
            tc.tile_pool(name="partp", bufs=2) as partp,
            tc.tile_pool(name="psfc", bufs=4, space="PSUM") as psfc,
            tc.tile_pool(name="psw", bufs=2, space="PSUM") as psw,
            tc.tile_pool(name="pss", bufs=2, space="PSUM") as pss,
            tc.tile_pool(name="dram", bufs=1, space="DRAM") as dram,
        ):
            ones1 = cst.tile([1, 128], BF16, tag="ones1")
            nc.vector.memset(ones1, 1.0)
            ones64 = cst.tile([64, 128], BF16, tag="ones64")
            nc.vector.memset(ones64, 1.0)

            # ~8us of dummy matmuls: HAM sees sustained PE activity and
            # releases the clock gate before fc1 arrives (K=8/8 from the
            # first real matmul instead of half-rate)
            warm_ps = psw.tile([128, 128], F32, tag="wide", name="warm_ps")
            for i in range(40):
                nc.tensor.matmul(warm_ps, ones64, ones64,
                                 start=(i == 0), stop=(i == 39))

            # ---------- sync queue: trunk inputs then the Wru stream ----------
            # x first, then w1 in 4 chunk-groups so fc1 can start early and
            # keep the PE continuously busy (HAM warm)
            xsb = cst.tile([128, KCH * B], BF16, tag="xsb")
            nc.sync.dma_start(xsb, xt)
            W1G = [(0, 7), (7, 7), (14, 6), (20, 6)]
            w1g = []
            for gi, (base, L) in enumerate(W1G):
                t = cst.tile([128, L * H], BF16, tag=f"w1g{gi}", name=f"w1g{gi}")
                nc.sync.dma_start(t, w1[:, base * H:(base + L) * H])
                w1g.append(t)

            # Wru stream + head pre-sum: pairwise TT-add tree on DVE (bf16
            # unit-stride TT hits the 2x_1P mode; tensor_reduce does not).
            # Supertile = 16 head-blocks of 500; tree halves 4 times.
            acc = [sb.tile([128, RPC], BF16, tag=f"acc{k}", name=f"acc{k}")
                   for k in range(4)]

            def tree4(wt, kc, g, h):
                # sum 4 head-blocks [128, 4*RPC] -> [128, RPC] into acc[kc]
                t3 = partp.tile([128, 2 * RPC], BF16, tag="t3",
                                name=f"t3_{kc}_{g}_{h}")
                nc.vector.tensor_add(t3, wt[:, 0:2 * RPC], wt[:, 2 * RPC:4 * RPC])
                if g == 0 and h == 0:
                    nc.vector.tensor_add(acc[kc], t3[:, 0:RPC], t3[:, RPC:2 * RPC])
                else:
                    part = partp.tile([128, RPC], BF16, tag="part",
                                      name=f"part{kc}_{g}_{h}")
                    nc.vector.tensor_add(part, t3[:, 0:RPC], t3[:, RPC:2 * RPC])
                    nc.vector.tensor_add(acc[kc], acc[kc], part)

            def tree8(wt, kc, g, h):
                # sum 8 head-blocks [128, 8*RPC] -> [128, RPC] into acc[kc]
                t2 = partp.tile([128, 4 * RPC], BF16, tag="t2",
                                name=f"t2_{kc}_{g}_{h}")
                nc.vector.tensor_add(t2, wt[:, 0:4 * RPC], wt[:, 4 * RPC:8 * RPC])
                tree4(t2, kc, g, h)

            for kc in range(4):
                for g in range(NGRP):
                    last = (kc == 3 and g == NGRP - 1)
                    if not last:
                        wt = wrupool.tile([128, GRP * RPC], BF16, tag="wru",
                                          name=f"wru_t{kc}_{g}")
                        nc.sync.dma_start(wt, wrup[kc, g])
                        t1 = partp.tile([128, 8 * RPC], BF16, tag="t1",
                                        name=f"t1_{kc}_{g}")
                        nc.vector.tensor_add(t1, wt[:, 0:8 * RPC],
                                             wt[:, 8 * RPC:16 * RPC])
                        tree8(t1, kc, g, 0)
                    else:
                        # split the final supertile into quarters so the
                        # post-stream DVE lag is one quarter-tree; quarters
                        # share one regular pool slot (no extra SBUF)
                        wt = wrupool.tile([128, GRP * RPC], BF16, tag="wru",
                                          name=f"wru_t{kc}_{g}")
                        for h in range(4):
                            nc.sync.dma_start(
                                wt[:, h * 4 * RPC:(h + 1) * 4 * RPC],
                                wrup[kc, g][:, h * 4 * RPC:(h + 1) * 4 * RPC])
                            tree4(wt[:, h * 4 * RPC:(h + 1) * 4 * RPC],
                                  kc, g, h)

            # ---------- scalar queue: small loads ----------
            b1sb = cst.tile([128, 4], F32, tag="b1sb")
            nc.scalar.dma_start(b1sb, b1p)
            b2sb = cst.tile([128, 4], F32, tag="b2sb")
            nc.scalar.dma_start(b2sb, b2p)
            w2sb = cst.tile([128, 4 * H], BF16, tag="w2sb")
            nc.scalar.dma_start(w2sb, w2p)
            wacsb = cst.tile([128, 4 * RPC], BF16, tag="wacsb")
            nc.scalar.dma_start(wacsb, wacp)
            bacsb = cst.tile([1, RPC], BF16, tag="bacsb")
            nc.scalar.dma_start(bacsb, bacp)
            augsb = cst.tile([128, 4 * AUG], BF16, tag="augsb")
            nc.scalar.dma_start(augsb, augp)
            augbsb = cst.tile([1, AUG], BF16, tag="augbsb")
            nc.scalar.dma_start(augbsb, augb)
            brusb = cst.tile([64, RPC], BF16, tag="brusb")
            nc.scalar.dma_start(brusb, brup)

            # ---------- fc1: h1T[ht] = sum_k W1[k, ht]·xT[k, b] ----------
            # kc-outer so matmuls start as soon as x + the first w1 group land
            ps1 = [psfc.tile([128, B], F32, tag="fc", name=f"ps1_{ht}")
                   for ht in range(4)]
            for gi, (base, L) in enumerate(W1G):
                for j in range(L):
                    kc = base + j
                    for ht in range(4):
                        nc.tensor.matmul(
                            ps1[ht],
                            w1g[gi][:, j * H + ht * 128:j * H + (ht + 1) * 128],
                            xsb[:, ts(kc, B)],
                            start=(kc == 0), stop=(kc == KCH - 1))
            h1loc = sb.tile([128, 4, B], BF16, tag="h1loc")
            for ht in range(4):
                nc.scalar.copy(h1loc[:, ht, :], ps1[ht])

            # ---------- fc1 cross-core reduction: ONE AllReduce (bf16).
            # Measured: AllReduce rides the CCE datapath and does NOT starve
            # the model DMA queues; AllToAll/ReduceScatter stall the Wru
            # stream for ~25-30us. AR also gives every core the full sum, so
            # no AllGather is needed at all.
            ar_din = dram.tile([128, 4, B], BF16, tag="ar_din")
            ar_dout = dram.tile([128, 4, B], BF16, tag="ar_dout",
                                addr_space="Shared")
            nc.scalar.dma_start(ar_din, h1loc)
            nc.gpsimd.collective_compute(
                "AllReduce", ADD,
                replica_groups=[list(range(NC))],
                ins=[ar_din.opt()], outs=[ar_dout.opt()],
            )
            h1r = sb.tile([128, 4, B], BF16, tag="h1r")
            nc.scalar.dma_start(h1r, ar_dout)
            h1T = []
            for ht in range(4):
                t = sb.tile([128, B], BF16, tag=f"h1T{ht}", name=f"h1T{ht}")
                nc.scalar.activation(t, h1r[:, ht, :], RELU,
                                     bias=b1sb[:, ht:ht + 1])
                h1T.append(t)

            # ---------- fc2 (redundant on every core, trivial) ----------
            hT = []
            for ht in range(4):
                p = psfc.tile([128, B], F32, tag="fc", name=f"ps2_{ht}")
                for kc in range(4):
                    nc.tensor.matmul(p, w2sb[:, kc * H + ht * 128:kc * H + (ht + 1) * 128],
                                     h1T[kc],
                                     start=(kc == 0), stop=(kc == 3))
                t = sb.tile([128, B], BF16, tag=f"hT{ht}", name=f"hT{ht}")
                nc.scalar.activation(t, p, RELU, bias=b2sb[:, ht:ht + 1])
                hT.append(t)

            # ---------- assoc heads + value_c (augmented matmul) ----------
            # row-sums for both the cache head (cols 0,1) and the rec head
            # (cols 2,3) gather in one tile -> ONE tail AllGather
            ar_in = sb.tile([128, 4], F32, tag="ar_in")
            value_sb = []
            junkA = sb.tile([128, M], F32, tag="junkA")
            for bt in range(2):
                psA = psw.tile([128, AUG], F32, tag="wide", name=f"psA{bt}")
                nc.tensor.matmul(psA, ones1, augbsb, start=True, stop=False)
                for kc in range(4):
                    nc.tensor.matmul(psA, hT[kc][:, ts(bt, 128)],
                                     augsb[:, kc * AUG:(kc + 1) * AUG],
                                     start=False, stop=(kc == 3))
                psA_sb = sb.tile([128, AUG], F32, tag=f"psAsb{bt}", name=f"psAsb{bt}")
                nc.scalar.copy(psA_sb, psA)
                advs = psA_sb[:, 0:HPC * (M + 1)].rearrange("p (n u) -> p n u", u=M + 1)
                negm = sb.tile([128, HPC], F32, tag=f"negmA{bt}", name=f"negmA{bt}")
                for n in range(HPC):
                    nc.scalar.activation(junkA, advs[:, n, 0:M], COPY,
                                         scale=-1.0 / M,
                                         accum_out=negm[:, n:n + 1])
                tmp = sb.tile([128, HPC], F32, tag=f"tmpA{bt}", name=f"tmpA{bt}")
                nc.gpsimd.tensor_add(tmp, advs[:, :, M], negm)
                q = sb.tile([128, HPC * M], F32, tag=f"qA{bt}", name=f"qA{bt}")
                nc.gpsimd.tensor_tensor(
                    out=q.rearrange("p (n m) -> p n m", m=M),
                    in0=advs[:, :, 0:M],
                    in1=tmp.broadcast_to([128, HPC, M]),
                    op=ADD)
                nc.scalar.dma_start(out_assoc[ts(bt, 128), :], q)
                value_sb.append(psA_sb[:, AUG - 1:AUG])

            # ---------- cache head (R-slice) + early row-sums ----------
            adv_c_sb = []
            for bt in range(2):
                psC = psw.tile([128, RPC], F32, tag="wide", name=f"psC{bt}")
                nc.tensor.matmul(psC, ones1, bacsb, start=True, stop=False)
                for kc in range(4):
                    nc.tensor.matmul(psC, hT[kc][:, ts(bt, 128)],
                                     wacsb[:, ts(kc, RPC)],
                                     start=False, stop=(kc == 3))
                t = sb.tile([128, RPC], F32, tag=f"advc{bt}", name=f"advc{bt}")
                nc.scalar.activation(t, psC, COPY,
                                     accum_out=ar_in[:, bt:bt + 1])
                adv_c_sb.append(t)

            # ---------- S = hT.T @ acc (+ sum_n bru), interleaved by kc ------
            psS = []
            for bt in range(2):
                p = pss.tile([128, RPC], F32, tag="s", name=f"psS{bt}")
                nc.tensor.matmul(p, ones64, brusb, start=True, stop=False)
                psS.append(p)
            for kc in range(4):
                for bt in range(2):
                    nc.tensor.matmul(psS[bt], hT[kc][:, ts(bt, 128)], acc[kc],
                                     start=False, stop=(kc == 3))
            # S row-sums straight from PSUM on the (now idle) DVE
            for bt in range(2):
                nc.vector.tensor_reduce(ar_in[:, 2 + bt:3 + bt], psS[bt],
                                        axis=mybir.AxisListType.X, op=ADD)

            # single tail AllGather of all four row-sum columns
            ag_din = dram.tile([128, 4], F32, tag="ag_din")
            ag_dout = dram.tile([NC * 128, 4], F32, tag="ag_dout",
                                addr_space="Shared")
            nc.scalar.dma_start(ag_din, ar_in)
            nc.gpsimd.collective_compute(
                "AllGather", mybir.AluOpType.bypass,
                replica_groups=[list(range(NC))],
                ins=[ag_din.opt()], outs=[ag_dout.opt()],
            )
            rall = sb.tile([128, NC * 4], F32, tag="rall")
            nc.scalar.dma_start(rall, ag_dout.rearrange("(g p) c -> p g c", p=128))
            rview = bass.AP(rall.tensor, rall.offset,
                            [rall.ap[0], [1, 4], [4, NC]])
            gsum = sb.tile([128, 4], F32, tag="gsum")
            nc.vector.tensor_reduce(gsum, rview, axis=mybir.AxisListType.X, op=ADD)
            negm = sb.tile([128, 4], F32, tag="negm")
            nc.scalar.activation(negm, gsum, COPY, scale=-1.0 / R)
            # qr on DVE straight from PSUM, qc on ACT — in parallel; each
            # output goes out as ONE DMA (both batch halves) to halve the
            # dispatch + receipt chains, split across the two HWDGE queues
            qrb = sb.tile([128, 2 * RPC], BF16, tag="qrb")
            qcb = sb.tile([128, 2 * RPC], BF16, tag="qcb")
            for bt in range(2):
                nc.vector.tensor_scalar(out=qrb[:, bt * RPC:(bt + 1) * RPC],
                                        in0=psS[bt],
                                        scalar1=negm[:, 2 + bt:3 + bt],
                                        scalar2=None, op0=ADD)
                vm = sb.tile([128, 1], F32, tag=f"vm{bt}", name=f"vm{bt}")
                nc.gpsimd.tensor_add(vm, value_sb[bt], negm[:, bt:bt + 1])
                nc.scalar.activation(qcb[:, bt * RPC:(bt + 1) * RPC],
                                     adv_c_sb[bt], IDENT, bias=vm, scale=1.0)
            nc.sync.dma_start(out_rec.rearrange("(t p) r -> p t r", p=128),
                              qrb.rearrange("p (t r) -> p t r", r=RPC))
            nc.scalar.dma_start(out_cache.rearrange("(t p) r -> p t r", p=128),
                                qcb.rearrange("p (t r) -> p t r", r=RPC))

    nc.compile()
    return nc


_CACHED = None


def _get_program():
    global _CACHED
    if _CACHED is None:
        _CACHED = build_program()
    return _CACHED


def make_in_maps(x, W1, b1, W2, b2, Wvc, bvc, Wac, bac, Wvu, bvu, Wau, bau, Wru, bru):
    f = np.float32
    x_bf = np.asarray(x, f).astype(BF)                    # [B, STATE]
    W1_bf = np.asarray(W1, f).astype(BF)                  # [STATE, H]
    W2_bf = np.asarray(W2, f).astype(BF)
    Wac_bf = np.asarray(Wac, f).astype(BF)
    Wru_bf = np.asarray(Wru, f).astype(BF)                # [64, 512, 4000]
    Wau_f = np.asarray(Wau, f)
    Wvu_f = np.asarray(Wvu, f)
    Wvc_f = np.asarray(Wvc, f).reshape(H)
    bau_f = np.asarray(bau, f)
    bvu_f = np.asarray(bvu, f)
    bvc_f = np.asarray(bvc, f).reshape(1)
    bru_bf = np.asarray(bru, f).astype(BF)
    bac_f = np.asarray(bac, f)
    b1_f = np.asarray(b1, f)
    b2_f = np.asarray(b2, f)

    # w2: [h1, h2] -> [p, kc*H + h2]
    w2p = np.ascontiguousarray(
        W2_bf.reshape(4, 128, H).transpose(1, 0, 2)).reshape(128, 4 * H)
    b1p = np.ascontiguousarray(b1_f.reshape(4, 128).T)
    b2p = np.ascontiguousarray(b2_f.reshape(4, 128).T)

    in_maps = []
    for c in range(NC):
        k0 = c * KPC_RAW
        r0 = c * RPC
        h0 = c * HPC
        # xT: [p, kc*B + b] = x[b, k0 + kc*128 + p]
        xs = np.zeros((KPC, B), BF)
        xs[:KPC_RAW] = x_bf[:, k0:k0 + KPC_RAW].T
        xt = np.ascontiguousarray(
            xs.reshape(KCH, 128, B).transpose(1, 0, 2)).reshape(128, KCH * B)
        # w1: [p, kc*H + h] = W1[k0 + kc*128 + p, h]
        w1s = np.zeros((KPC, H), BF)
        w1s[:KPC_RAW] = W1_bf[k0:k0 + KPC_RAW]
        w1p_ = np.ascontiguousarray(
            w1s.reshape(KCH, 128, H).transpose(1, 0, 2)).reshape(128, KCH * H)
        # wac: [p, kc*RPC + r] = Wac[kc*128 + p, r0 + r]
        wacp_ = np.ascontiguousarray(
            Wac_bf[:, r0:r0 + RPC].reshape(4, 128, RPC).transpose(1, 0, 2)
        ).reshape(128, 4 * RPC)
        # wru supertiles, head-block layout (contiguous blocks for the DVE
        # TT tree): [kc, g, p, blk*RPC + r] = Wru[g*GRP+blk, kc*128+p, r0+r]
        ws = Wru_bf[:, :, r0:r0 + RPC]                    # [64, 512, 500]
        a = ws.reshape(NGRP, GRP, 4, 128, RPC)            # [g, blk, kc, p, r]
        wru_p = np.ascontiguousarray(a.transpose(2, 0, 3, 1, 4)).reshape(
            4, NGRP, 128, GRP * RPC)
        # aug weights: [k, n*(M+1)+m | n*(M+1)+M | 88]
        aug_full = np.empty((H, AUG), f)
        aug_full[:, 0:HPC * (M + 1)] = np.concatenate(
            [Wau_f[h0:h0 + HPC].transpose(1, 0, 2),            # [H, 8, 10]
             Wvu_f[h0:h0 + HPC].T[:, :, None]], axis=2         # [H, 8, 1]
        ).reshape(H, HPC * (M + 1))
        aug_full[:, AUG - 1] = Wvc_f
        augp_ = np.ascontiguousarray(
            aug_full.astype(BF).reshape(4, 128, AUG).transpose(1, 0, 2)
        ).reshape(128, 4 * AUG)
        augb_ = np.empty((1, AUG), f)
        augb_[0, 0:HPC * (M + 1)] = np.concatenate(
            [bau_f[h0:h0 + HPC], bvu_f[h0:h0 + HPC, None]], axis=1
        ).reshape(HPC * (M + 1))
        augb_[0, AUG - 1] = bvc_f[0]
        m = {
            "xt": xt,
            "w1": w1p_,
            "b1p": b1p,
            "w2p": w2p,
            "b2p": b2p,
            "wacp": wacp_,
            "bacp": np.ascontiguousarray(bac_f[None, r0:r0 + RPC]).astype(BF),
            "wrup": wru_p,
            "brup": np.ascontiguousarray(bru_bf[:, r0:r0 + RPC]),
            "augp": augp_,
            "augb": augb_.astype(BF),
        }
        in_maps.append(m)
    return in_maps


def assemble(results):
    q = np.empty((B, 2 * R + NH * M), np.float32)
    for c in range(NC):
        r0 = c * RPC
        a0 = c * HPC * M
        q[:, r0:r0 + RPC] = results[c]["out_cache"]
        q[:, R + r0:R + r0 + RPC] = results[c]["out_rec"]
        q[:, 2 * R + a0:2 * R + a0 + HPC * M] = results[c]["out_assoc"]
    return q


def run(in_maps, **kw):
    nc = _get_program()
    return bass_utils.run_bass_kernel_spmd(nc, in_maps, core_ids=list(range(NC)), **kw)


def kernel(**inputs):
    in_maps = make_in_maps(**{k: np.asarray(v) for k, v in inputs.items()})
    res = run(in_maps)
    return assemble(res.results)


# revision 30
# speedup vs baseline: 1.0357x; 1.0357x over previous
"""Trainium2 Bass kernel for nn_MultiHeadDuelingDQN (8-core SPMD), v2.

Model (B=256, STATE=26240, H=512, R=4000, N=64 heads, M=10):
    h  = relu(relu(x@W1+b1)@W2+b2)
    q_cache = h@Wvc+bvc + (h@Wac+bac) - mean_R(h@Wac+bac)
    q_assoc = per-head dueling over M (local means)
    q_rec   = S - mean_R(S),  S = sum_n (h@Wru[n]+bru[n])   [exact rewrite:
              rec_global has zero row-mean so the second mean is a no-op]

Design (v8; baseline v1 was ~344us, this is ~145us/core + launch skew):
  - Everything streams and computes in bf16 (halves HBM traffic, 4x matmul
    rate vs fp32); fp32 only in PSUM accumulation and final means.
  - All weights host-repacked into exact SBUF images [128, X]: every load is
    one contiguous-per-partition DMA; ZERO device transposes. Trunk computes
    h1T/h2T [h, b] directly (lhsT = natural W1/W2 k-chunks, rhs = host-
    transposed xT); heads consume h2T as lhsT. b1/b2 are per-partition ACT
    biases fused into relu+cast.
  - Wru head-sum: DVE pairwise TT-add tree over 16-head-block supertiles
    [128, 8000] bf16 (unit-stride bf16 TT hits the DVE 2x_1P mode;
    tensor_reduce does not). DVE does nothing else during the stream. The
    last supertile is split in half to shrink the post-stream DVE lag.
  - fc1 contraction-sharded; partials combined with ONE bf16 AllReduce.
    Measured on this fabric: AllReduce rides the CCE datapath and does NOT
    starve the model DMA queues, while AllToAll/ReduceScatter stall the Wru
    stream ~25-30us; AR output is the full sum so no AllGather is needed.
    fc2 + all heads run redundantly/locally (trivial vs the stream).
  - Full-R row-means (cache+rec) share ONE tiny tail AllGather; the rec
    finalize (qr) runs on DVE straight from PSUM while ACT does qc, and tail
    output DMAs go on the idle sync queue.
  - psS accumulation interleaved by k-chunk so only the last acc[3] matmul
    waits for the stream end.

kernel(**inputs) takes full unsharded fp32 inputs, returns full [256, 8640].
"""
import os
os.environ.setdefault("NEURON_RT_DBG_RDH_CC", "0")

import numpy as np
import ml_dtypes

import concourse.bass as bass
import concourse.mybir as mybir
import concourse.tile as tile
from concourse import bacc
from concourse import bass_utils
from concourse.bass import ts

NC = 8
B, H, STATE, R, NH, M = 256, 512, 26240, 4000, 64, 10
KPC_RAW = STATE // NC          # 3280
KCH = 26                       # k-chunks of 128 per core (padded)
KPC = KCH * 128                # 3328
RPC = R // NC                  # 500
HPC = NH // NC                 # 8 heads per core
AUG = HPC * (M + 1) + 1        # 89 = [8x(10 adv + 1 val)] + value_c
GRP = 16                       # heads per wru supertile
NGRP = NH // GRP               # 4 supertiles per k-chunk
F32 = mybir.dt.float32
BF16 = mybir.dt.bfloat16
RELU = mybir.ActivationFunctionType.Relu
COPY = mybir.ActivationFunctionType.Copy
IDENT = mybir.ActivationFunctionType.Identity
ADD = mybir.AluOpType.add
BF = ml_dtypes.bfloat16


def build_program(wru_bufs=6):
    nc = bacc.Bacc("TRN2", target_bir_lowering=False, debug=False, num_devices=NC)

    # ---- per-core I/O (all host-packed to exact SBUF images) ----
    xt = nc.dram_tensor("xt", [128, KCH * B], BF16, kind="ExternalInput").ap()
    w1 = nc.dram_tensor("w1", [128, KCH * H], BF16, kind="ExternalInput").ap()
    b1p = nc.dram_tensor("b1p", [128, 4], F32, kind="ExternalInput").ap()
    w2p = nc.dram_tensor("w2p", [128, 4 * H], BF16, kind="ExternalInput").ap()
    b2p = nc.dram_tensor("b2p", [128, 4], F32, kind="ExternalInput").ap()
    wacp = nc.dram_tensor("wacp", [128, 4 * RPC], BF16, kind="ExternalInput").ap()
    bacp = nc.dram_tensor("bacp", [1, RPC], BF16, kind="ExternalInput").ap()
    # supertiles: [kc, g, p, r*GRP+gi] = Wru[g*GRP+gi, kc*128+p, r0+r]
    wrup = nc.dram_tensor("wrup", [4, NGRP, 128, GRP * RPC], BF16,
                          kind="ExternalInput").ap()
    brup = nc.dram_tensor("brup", [NH, RPC], BF16, kind="ExternalInput").ap()
    augp = nc.dram_tensor("augp", [128, 4 * AUG], BF16, kind="ExternalInput").ap()
    augb = nc.dram_tensor("augb", [1, AUG], BF16, kind="ExternalInput").ap()

    out_cache = nc.dram_tensor("out_cache", [B, RPC], BF16, kind="ExternalOutput").ap()
    out_rec = nc.dram_tensor("out_rec", [B, RPC], BF16, kind="ExternalOutput").ap()
    out_assoc = nc.dram_tensor("out_assoc", [B, HPC * M], F32, kind="ExternalOutput").ap()

    with tile.TileContext(nc) as tc, \
         nc.allow_low_precision(reason="bf16 stream reduction; gate is 2e-2"):
        with (
            tc.tile_pool(name="cst", bufs=1) as cst,
            tc.tile_pool(name="sb", bufs=1) as sb,
            tc.tile_pool(name="wrupool", bufs=wru_bufs) as wrupool,
            tc.tile_pool(name="partp", bufs=2) as partp,
            tc.tile_pool(name="psfc", bufs=4, space="PSUM") as psfc,
            tc.tile_pool(name="psw", bufs=2, space="PSUM") as psw,
            tc.tile_pool(name="pss", bufs=2, space="PSUM") as pss,
            tc.tile_pool(name="dram", bufs=1, space="DRAM") as dram,
        ):
            ones1 = cst.tile([1, 128], BF16, tag="ones1")
            nc.vector.memset(ones1, 1.0)
            ones64 = cst.tile([64, 128], BF16, tag="ones64")
            nc.vector.memset(ones64, 1.0)

            # ~8us of dummy matmuls: HAM sees sustained PE activity and
            # releases the clock gate before fc1 arrives (K=8/8 from the
            # first real matmul instead of half-rate)
            warm_ps = psw.tile([128, 128], F32, tag="wide", name="warm_ps")
            for i in range(40):
                nc.tensor.matmul(warm_ps, ones64, ones64,
                                 start=(i == 0), stop=(i == 39))

            # ---------- trunk inputs ----------
            # x + the first w1 group ride the sync queue ahead of the Wru
            # stream (fc1 can start ~15us in); the remaining w1 groups move
            # on the scalar queue so the critical stream queue carries 3.5MB
            # less. fc1 is kc-outer, so it consumes groups in order as they
            # land.
            xsb = cst.tile([128, KCH * B], BF16, tag="xsb")
            nc.sync.dma_start(xsb, xt)
            W1G = [(0, 7), (7, 7), (14, 6), (20, 6)]
            w1g = []
            for gi, (base, L) in enumerate(W1G):
                t = cst.tile([128, L * H], BF16, tag=f"w1g{gi}", name=f"w1g{gi}")
                eng = nc.sync if gi == 0 else nc.scalar
                eng.dma_start(t, w1[:, base * H:(base + L) * H])
                w1g.append(t)

            # Wru stream + head pre-sum: pairwise TT-add tree on DVE (bf16
            # unit-stride TT hits the 2x_1P mode; tensor_reduce does not).
            # Supertile = 16 head-blocks of 500; tree halves 4 times.
            acc = [sb.tile([128, RPC], BF16, tag=f"acc{k}", name=f"acc{k}")
                   for k in range(4)]

            def tree4(wt, kc, g, h):
                # sum 4 head-blocks [128, 4*RPC] -> [128, RPC] into acc[kc]
                t3 = partp.tile([128, 2 * RPC], BF16, tag="t3",
                                name=f"t3_{kc}_{g}_{h}")
                nc.vector.tensor_add(t3, wt[:, 0:2 * RPC], wt[:, 2 * RPC:4 * RPC])
                if g == 0 and h == 0:
                    nc.vector.tensor_add(acc[kc], t3[:, 0:RPC], t3[:, RPC:2 * RPC])
                else:
                    part = partp.tile([128, RPC], BF16, tag="part",
                                      name=f"part{kc}_{g}_{h}")
                    nc.vector.tensor_add(part, t3[:, 0:RPC], t3[:, RPC:2 * RPC])
                    nc.vector.tensor_add(acc[kc], acc[kc], part)

            def tree8(wt, kc, g, h):
                # sum 8 head-blocks [128, 8*RPC] -> [128, RPC] into acc[kc]
                t2 = partp.tile([128, 4 * RPC], BF16, tag="t2",
                                name=f"t2_{kc}_{g}_{h}")
                nc.vector.tensor_add(t2, wt[:, 0:4 * RPC], wt[:, 4 * RPC:8 * RPC])
                tree4(t2, kc, g, h)

            for kc in range(4):
                for g in range(NGRP):
                    last = (kc == 3 and g == NGRP - 1)
                    if not last:
                        wt = wrupool.tile([128, GRP * RPC], BF16, tag="wru",
                                          name=f"wru_t{kc}_{g}")
                        nc.sync.dma_start(wt, wrup[kc, g])
                        t1 = partp.tile([128, 8 * RPC], BF16, tag="t1",
                                        name=f"t1_{kc}_{g}")
                        nc.vector.tensor_add(t1, wt[:, 0:8 * RPC],
                                             wt[:, 8 * RPC:16 * RPC])
                        tree8(t1, kc, g, 0)
                    else:
                        # split the final supertile into quarters so the
                        # post-stream DVE lag is one quarter-tree; quarters
                        # share one regular pool slot (no extra SBUF)
                        wt = wrupool.tile([128, GRP * RPC], BF16, tag="wru",
                                          name=f"wru_t{kc}_{g}")
                        for h in range(4):
                            nc.sync.dma_start(
                                wt[:, h * 4 * RPC:(h + 1) * 4 * RPC],
                                wrup[kc, g][:, h * 4 * RPC:(h + 1) * 4 * RPC])
                            tree4(wt[:, h * 4 * RPC:(h + 1) * 4 * RPC],
                                  kc, g, h)

            # ---------- scalar queue: small loads ----------
            b1sb = cst.tile([128, 4], F32, tag="b1sb")
            nc.scalar.dma_start(b1sb, b1p)
            b2sb = cst.tile([128, 4], F32, tag="b2sb")
            nc.scalar.dma_start(b2sb, b2p)
            w2sb = cst.tile([128, 4 * H], BF16, tag="w2sb")
            nc.scalar.dma_start(w2sb, w2p)
            wacsb = cst.tile([128, 4 * RPC], BF16, tag="wacsb")
            nc.scalar.dma_start(wacsb, wacp)
            bacsb = cst.tile([1, RPC], BF16, tag="bacsb")
            nc.scalar.dma_start(bacsb, bacp)
            augsb = cst.tile([128, 4 * AUG], BF16, tag="augsb")
            nc.scalar.dma_start(augsb, augp)
            augbsb = cst.tile([1, AUG], BF16, tag="augbsb")
            nc.scalar.dma_start(augbsb, augb)
            brusb = cst.tile([64, RPC], BF16, tag="brusb")
            nc.scalar.dma_start(brusb, brup)

            # ---------- fc1: h1T[ht] = sum_k W1[k, ht]·xT[k, b] ----------
            # kc-outer so matmuls start as soon as x + the first w1 group land
            ps1 = [psfc.tile([128, B], F32, tag="fc", name=f"ps1_{ht}")
                   for ht in range(4)]
            for gi, (base, L) in enumerate(W1G):
                for j in range(L):
                    kc = base + j
                    for ht in range(4):
                        nc.tensor.matmul(
                            ps1[ht],
                            w1g[gi][:, j * H + ht * 128:j * H + (ht + 1) * 128],
                            xsb[:, ts(kc, B)],
                            start=(kc == 0), stop=(kc == KCH - 1))
            h1loc = sb.tile([128, 4, B], BF16, tag="h1loc")
            for ht in range(4):
                nc.scalar.copy(h1loc[:, ht, :], ps1[ht])

            # ---------- fc1 cross-core reduction: ONE AllReduce (bf16).
            # Measured: AllReduce rides the CCE datapath and does NOT starve
            # the model DMA queues; AllToAll/ReduceScatter stall the Wru
            # stream for ~25-30us. AR also gives every core the full sum, so
            # no AllGather is needed at all.
            ar_din = dram.tile([128, 4, B], BF16, tag="ar_din")
            ar_dout = dram.tile([128, 4, B], BF16, tag="ar_dout",
                                addr_space="Shared")
            nc.scalar.dma_start(ar_din, h1loc)
            nc.gpsimd.collective_compute(
                "AllReduce", ADD,
                replica_groups=[list(range(NC))],
                ins=[ar_din.opt()], outs=[ar_dout.opt()],
            )
            h1r = sb.tile([128, 4, B], BF16, tag="h1r")
            nc.scalar.dma_start(h1r, ar_dout)
            h1T = []
            for ht in range(4):
                t = sb.tile([128, B], BF16, tag=f"h1T{ht}", name=f"h1T{ht}")
                nc.scalar.activation(t, h1r[:, ht, :], RELU,
                                     bias=b1sb[:, ht:ht + 1])
                h1T.append(t)

            # ---------- fc2 (redundant on every core, trivial) ----------
            hT = []
            for ht in range(4):
                p = psfc.tile([128, B], F32, tag="fc", name=f"ps2_{ht}")
                for kc in range(4):
                    nc.tensor.matmul(p, w2sb[:, kc * H + ht * 128:kc * H + (ht + 1) * 128],
                                     h1T[kc],
                                     start=(kc == 0), stop=(kc == 3))
                t = sb.tile([128, B], BF16, tag=f"hT{ht}", name=f"hT{ht}")
                nc.scalar.activation(t, p, RELU, bias=b2sb[:, ht:ht + 1])
                hT.append(t)

            # ---------- assoc heads + value_c (augmented matmul) ----------
            # row-sums for both the cache head (cols 0,1) and the rec head
            # (cols 2,3) gather in one tile -> ONE tail AllGather
            ar_in = sb.tile([128, 4], F32, tag="ar_in")
            value_sb = []
            junkA = sb.tile([128, M], F32, tag="junkA")
            for bt in range(2):
                psA = psw.tile([128, AUG], F32, tag="wide", name=f"psA{bt}")
                nc.tensor.matmul(psA, ones1, augbsb, start=True, stop=False)
                for kc in range(4):
                    nc.tensor.matmul(psA, hT[kc][:, ts(bt, 128)],
                                     augsb[:, kc * AUG:(kc + 1) * AUG],
                                     start=False, stop=(kc == 3))
                psA_sb = sb.tile([128, AUG], F32, tag=f"psAsb{bt}", name=f"psAsb{bt}")
                nc.scalar.copy(psA_sb, psA)
                advs = psA_sb[:, 0:HPC * (M + 1)].rearrange("p (n u) -> p n u", u=M + 1)
                negm = sb.tile([128, HPC], F32, tag=f"negmA{bt}", name=f"negmA{bt}")
                for n in range(HPC):
                    nc.scalar.activation(junkA, advs[:, n, 0:M], COPY,
                                         scale=-1.0 / M,
                                         accum_out=negm[:, n:n + 1])
                tmp = sb.tile([128, HPC], F32, tag=f"tmpA{bt}", name=f"tmpA{bt}")
                nc.gpsimd.tensor_add(tmp, advs[:, :, M], negm)
                q = sb.tile([128, HPC * M], F32, tag=f"qA{bt}", name=f"qA{bt}")
                nc.gpsimd.tensor_tensor(
                    out=q.rearrange("p (n m) -> p n m", m=M),
                    in0=advs[:, :, 0:M],
                    in1=tmp.broadcast_to([128, HPC, M]),
                    op=ADD)
                nc.scalar.dma_start(out_assoc[ts(bt, 128), :], q)
                value_sb.append(psA_sb[:, AUG - 1:AUG])

            # ---------- cache head (R-slice) + early row-sums ----------
            adv_c_sb = []
            for bt in range(2):
                psC = psw.tile([128, RPC], F32, tag="wide", name=f"psC{bt}")
                nc.tensor.matmul(psC, ones1, bacsb, start=True, stop=False)
                for kc in range(4):
                    nc.tensor.matmul(psC, hT[kc][:, ts(bt, 128)],
                                     wacsb[:, ts(kc, RPC)],
                                     start=False, stop=(kc == 3))
                t = sb.tile([128, RPC], F32, tag=f"advc{bt}", name=f"advc{bt}")
                nc.scalar.activation(t, psC, COPY,
                                     accum_out=ar_in[:, bt:bt + 1])
                adv_c_sb.append(t)

            # ---------- S = hT.T @ acc (+ sum_n bru), interleaved by kc ------
            psS = []
            for bt in range(2):
                p = pss.tile([128, RPC], F32, tag="s", name=f"psS{bt}")
                nc.tensor.matmul(p, ones64, brusb, start=True, stop=False)
                psS.append(p)
            for kc in range(4):
                for bt in range(2):
                    nc.tensor.matmul(psS[bt], hT[kc][:, ts(bt, 128)], acc[kc],
                                     start=False, stop=(kc == 3))
            # S row-sums straight from PSUM on the (now idle) DVE
            for bt in range(2):
                nc.vector.tensor_reduce(ar_in[:, 2 + bt:3 + bt], psS[bt],
                                        axis=mybir.AxisListType.X, op=ADD)

            # single tail AllGather of all four row-sum columns
            ag_din = dram.tile([128, 4], F32, tag="ag_din")
            ag_dout = dram.tile([NC * 128, 4], F32, tag="ag_dout",
                                addr_space="Shared")
            nc.scalar.dma_start(ag_din, ar_in)
            nc.gpsimd.collective_compute(
                "AllGather", mybir.AluOpType.bypass,
                replica_groups=[list(range(NC))],
                ins=[ag_din.opt()], outs=[ag_dout.opt()],
            )
            rall = sb.tile([128, NC * 4], F32, tag="rall")
            nc.scalar.dma_start(rall, ag_dout.rearrange("(g p) c -> p g c", p=128))
            rview = bass.AP(rall.tensor, rall.offset,
                            [rall.ap[0], [1, 4], [4, NC]])
            gsum = sb.tile([128, 4], F32, tag="gsum")
            nc.vector.tensor_reduce(gsum, rview, axis=mybir.AxisListType.X, op=ADD)
            negm = sb.tile([128, 4], F32, tag="negm")
            nc.scalar.activation(negm, gsum, COPY, scale=-1.0 / R)
            # qr on DVE straight from PSUM, qc on ACT — in parallel; each
            # output goes out as ONE DMA (both batch halves) to halve the
            # dispatch + receipt chains, split across the two HWDGE queues
            qrb = sb.tile([128, 2 * RPC], BF16, tag="qrb")
            qcb = sb.tile([128, 2 * RPC], BF16, tag="qcb")
            for bt in range(2):
                nc.vector.tensor_scalar(out=qrb[:, bt * RPC:(bt + 1) * RPC],
                                        in0=psS[bt],
                                        scalar1=negm[:, 2 + bt:3 + bt],
                                        scalar2=None, op0=ADD)
                vm = sb.tile([128, 1], F32, tag=f"vm{bt}", name=f"vm{bt}")
                nc.gpsimd.tensor_add(vm, value_sb[bt], negm[:, bt:bt + 1])
                nc.scalar.activation(qcb[:, bt * RPC:(bt + 1) * RPC],
                                     adv_c_sb[bt], IDENT, bias=vm, scale=1.0)
            nc.sync.dma_start(out_rec.rearrange("(t p) r -> p t r", p=128),
                              qrb.rearrange("p (t r) -> p t r", r=RPC))
            nc.scalar.dma_start(out_cache.rearrange("(t p) r -> p t r", p=128),
                                qcb.rearrange("p (t r) -> p t r", r=RPC))

    nc.compile()
    return nc


_CACHED = None


def _get_program():
    global _CACHED
    if _CACHED is None:
        _CACHED = build_program()
    return _CACHED


def make_in_maps(x, W1, b1, W2, b2, Wvc, bvc, Wac, bac, Wvu, bvu, Wau, bau, Wru, bru):
    f = np.float32
    x_bf = np.asarray(x, f).astype(BF)                    # [B, STATE]
    W1_bf = np.asarray(W1, f).astype(BF)                  # [STATE, H]
    W2_bf = np.asarray(W2, f).astype(BF)
    Wac_bf = np.asarray(Wac, f).astype(BF)
    Wru_bf = np.asarray(Wru, f).astype(BF)                # [64, 512, 4000]
    Wau_f = np.asarray(Wau, f)
    Wvu_f = np.asarray(Wvu, f)
    Wvc_f = np.asarray(Wvc, f).reshape(H)
    bau_f = np.asarray(bau, f)
    bvu_f = np.asarray(bvu, f)
    bvc_f = np.asarray(bvc, f).reshape(1)
    bru_bf = np.asarray(bru, f).astype(BF)
    bac_f = np.asarray(bac, f)
    b1_f = np.asarray(b1, f)
    b2_f = np.asarray(b2, f)

    # w2: [h1, h2] -> [p, kc*H + h2]
    w2p = np.ascontiguousarray(
        W2_bf.reshape(4, 128, H).transpose(1, 0, 2)).reshape(128, 4 * H)
    b1p = np.ascontiguousarray(b1_f.reshape(4, 128).T)
    b2p = np.ascontiguousarray(b2_f.reshape(4, 128).T)

    in_maps = []
    for c in range(NC):
        k0 = c * KPC_RAW
        r0 = c * RPC
        h0 = c * HPC
        # xT: [p, kc*B + b] = x[b, k0 + kc*128 + p]
        xs = np.zeros((KPC, B), BF)
        xs[:KPC_RAW] = x_bf[:, k0:k0 + KPC_RAW].T
        xt = np.ascontiguousarray(
            xs.reshape(KCH, 128, B).transpose(1, 0, 2)).reshape(128, KCH * B)
        # w1: [p, kc*H + h] = W1[k0 + kc*128 + p, h]
        w1s = np.zeros((KPC, H), BF)
        w1s[:KPC_RAW] = W1_bf[k0:k0 + KPC_RAW]
        w1p_ = np.ascontiguousarray(
            w1s.reshape(KCH, 128, H).transpose(1, 0, 2)).reshape(128, KCH * H)
        # wac: [p, kc*RPC + r] = Wac[kc*128 + p, r0 + r]
        wacp_ = np.ascontiguousarray(
            Wac_bf[:, r0:r0 + RPC].reshape(4, 128, RPC).transpose(1, 0, 2)
        ).reshape(128, 4 * RPC)
        # wru supertiles, head-block layout (contiguous blocks for the DVE
        # TT tree): [kc, g, p, blk*RPC + r] = Wru[g*GRP+blk, kc*128+p, r0+r]
        ws = Wru_bf[:, :, r0:r0 + RPC]                    # [64, 512, 500]
        a = ws.reshape(NGRP, GRP, 4, 128, RPC)            # [g, blk, kc, p, r]
        wru_p = np.ascontiguousarray(a.transpose(2, 0, 3, 1, 4)).reshape(
            4, NGRP, 128, GRP * RPC)
        # aug weights: [k, n*(M+1)+m | n*(M+1)+M | 88]
        aug_full = np.empty((H, AUG), f)
        aug_full[:, 0:HPC * (M + 1)] = np.concatenate(
            [Wau_f[h0:h0 + HPC].transpose(1, 0, 2),            # [H, 8, 10]
             Wvu_f[h0:h0 + HPC].T[:, :, None]], axis=2         # [H, 8, 1]
        ).reshape(H, HPC * (M + 1))
        aug_full[:, AUG - 1] = Wvc_f
        augp_ = np.ascontiguousarray(
            aug_full.astype(BF).reshape(4, 128, AUG).transpose(1, 0, 2)
        ).reshape(128, 4 * AUG)
        augb_ = np.empty((1, AUG), f)
        augb_[0, 0:HPC * (M + 1)] = np.concatenate(
            [bau_f[h0:h0 + HPC], bvu_f[h0:h0 + HPC, None]], axis=1
        ).reshape(HPC * (M + 1))
        augb_[0, AUG - 1] = bvc_f[0]
        m = {
            "xt": xt,
            "w1": w1p_,
            "b1p": b1p,
            "w2p": w2p,
            "b2p": b2p,
            "wacp": wacp_,
            "bacp": np.ascontiguousarray(bac_f[None, r0:r0 + RPC]).astype(BF),
            "wrup": wru_p,
            "brup": np.ascontiguousarray(bru_bf[:, r0:r0 + RPC]),
            "augp": augp_,
            "augb": augb_.astype(BF),
        }
        in_maps.append(m)
    return in_maps


def assemble(results):
    q = np.empty((B, 2 * R + NH * M), np.float32)
    for c in range(NC):
        r0 = c * RPC
        a0 = c * HPC * M
        q[:, r0:r0 + RPC] = results[c]["out_cache"]
        q[:, R + r0:R + r0 + RPC] = results[c]["out_rec"]
        q[:, 2 * R + a0:2 * R + a0 + HPC * M] = results[c]["out_assoc"]
    return q


def run(in_maps, **kw):
    nc = _get_program()
    return bass_utils.run_bass_kernel_spmd(nc, in_maps, core_ids=list(range(NC)), **kw)


def kernel(**inputs):
    in_maps = make_in_maps(**{k: np.asarray(v) for k, v in inputs.items()})
    res = run(in_maps)
    return assemble(res.results)


# revision 31
# speedup vs baseline: 1.0603x; 1.0238x over previous
"""Trainium2 Bass kernel for nn_MultiHeadDuelingDQN (8-core SPMD), v2.

Model (B=256, STATE=26240, H=512, R=4000, N=64 heads, M=10):
    h  = relu(relu(x@W1+b1)@W2+b2)
    q_cache = h@Wvc+bvc + (h@Wac+bac) - mean_R(h@Wac+bac)
    q_assoc = per-head dueling over M (local means)
    q_rec   = S - mean_R(S),  S = sum_n (h@Wru[n]+bru[n])   [exact rewrite:
              rec_global has zero row-mean so the second mean is a no-op]

Design (v8; baseline v1 was ~344us, this is ~145us/core + launch skew):
  - Everything streams and computes in bf16 (halves HBM traffic, 4x matmul
    rate vs fp32); fp32 only in PSUM accumulation and final means.
  - All weights host-repacked into exact SBUF images [128, X]: every load is
    one contiguous-per-partition DMA; ZERO device transposes. Trunk computes
    h1T/h2T [h, b] directly (lhsT = natural W1/W2 k-chunks, rhs = host-
    transposed xT); heads consume h2T as lhsT. b1/b2 are per-partition ACT
    biases fused into relu+cast.
  - Wru head-sum: DVE pairwise TT-add tree over 16-head-block supertiles
    [128, 8000] bf16 (unit-stride bf16 TT hits the DVE 2x_1P mode;
    tensor_reduce does not). DVE does nothing else during the stream. The
    last supertile is split in half to shrink the post-stream DVE lag.
  - fc1 contraction-sharded; partials combined with ONE bf16 AllReduce.
    Measured on this fabric: AllReduce rides the CCE datapath and does NOT
    starve the model DMA queues, while AllToAll/ReduceScatter stall the Wru
    stream ~25-30us; AR output is the full sum so no AllGather is needed.
    fc2 + all heads run redundantly/locally (trivial vs the stream).
  - Full-R row-means (cache+rec) share ONE tiny tail AllGather; the rec
    finalize (qr) runs on DVE straight from PSUM while ACT does qc, and tail
    output DMAs go on the idle sync queue.
  - psS accumulation interleaved by k-chunk so only the last acc[3] matmul
    waits for the stream end.

kernel(**inputs) takes full unsharded fp32 inputs, returns full [256, 8640].
"""
import os
os.environ.setdefault("NEURON_RT_DBG_RDH_CC", "0")

import numpy as np
import ml_dtypes

import concourse.bass as bass
import concourse.mybir as mybir
import concourse.tile as tile
from concourse import bacc
from concourse import bass_utils
from concourse.bass import ts

NC = 8
B, H, STATE, R, NH, M = 256, 512, 26240, 4000, 64, 10
KPC_RAW = STATE // NC          # 3280
KCH = 26                       # k-chunks of 128 per core (padded)
KPC = KCH * 128                # 3328
RPC = R // NC                  # 500
HPC = NH // NC                 # 8 heads per core
AUG = HPC * (M + 1) + 1        # 89 = [8x(10 adv + 1 val)] + value_c
GRP = 16                       # heads per wru supertile
NGRP = NH // GRP               # 4 supertiles per k-chunk
F32 = mybir.dt.float32
BF16 = mybir.dt.bfloat16
RELU = mybir.ActivationFunctionType.Relu
COPY = mybir.ActivationFunctionType.Copy
IDENT = mybir.ActivationFunctionType.Identity
ADD = mybir.AluOpType.add
BF = ml_dtypes.bfloat16


def build_program(wru_bufs=6):
    nc = bacc.Bacc("TRN2", target_bir_lowering=False, debug=False, num_devices=NC)

    # ---- per-core I/O (all host-packed to exact SBUF images) ----
    xt = nc.dram_tensor("xt", [128, KCH * B], BF16, kind="ExternalInput").ap()
    w1 = nc.dram_tensor("w1", [128, KCH * H], BF16, kind="ExternalInput").ap()
    b1p = nc.dram_tensor("b1p", [128, 4], F32, kind="ExternalInput").ap()
    w2p = nc.dram_tensor("w2p", [128, 4 * H], BF16, kind="ExternalInput").ap()
    b2p = nc.dram_tensor("b2p", [128, 4], F32, kind="ExternalInput").ap()
    wacp = nc.dram_tensor("wacp", [128, 4 * RPC], BF16, kind="ExternalInput").ap()
    bacp = nc.dram_tensor("bacp", [1, RPC], BF16, kind="ExternalInput").ap()
    # supertiles: [kc, g, p, r*GRP+gi] = Wru[g*GRP+gi, kc*128+p, r0+r]
    wrup = nc.dram_tensor("wrup", [4, NGRP, 128, GRP * RPC], BF16,
                          kind="ExternalInput").ap()
    brup = nc.dram_tensor("brup", [NH, RPC], BF16, kind="ExternalInput").ap()
    augp = nc.dram_tensor("augp", [128, 4 * AUG], BF16, kind="ExternalInput").ap()
    augb = nc.dram_tensor("augb", [1, AUG], BF16, kind="ExternalInput").ap()

    out_cache = nc.dram_tensor("out_cache", [B, RPC], BF16, kind="ExternalOutput").ap()
    out_rec = nc.dram_tensor("out_rec", [B, RPC], BF16, kind="ExternalOutput").ap()
    out_assoc = nc.dram_tensor("out_assoc", [B, HPC * M], F32, kind="ExternalOutput").ap()

    with tile.TileContext(nc) as tc, \
         nc.allow_low_precision(reason="bf16 stream reduction; gate is 2e-2"):
        with (
            tc.tile_pool(name="cst", bufs=1) as cst,
            tc.tile_pool(name="sb", bufs=1) as sb,
            tc.tile_pool(name="wrupool", bufs=wru_bufs) as wrupool,
            tc.tile_pool(name="partp", bufs=2) as partp,
            tc.tile_pool(name="psfc", bufs=4, space="PSUM") as psfc,
            tc.tile_pool(name="psw", bufs=2, space="PSUM") as psw,
            tc.tile_pool(name="pss", bufs=2, space="PSUM") as pss,
            tc.tile_pool(name="dram", bufs=1, space="DRAM") as dram,
        ):
            ones1 = cst.tile([1, 128], BF16, tag="ones1")
            nc.vector.memset(ones1, 1.0)
            ones64 = cst.tile([64, 128], BF16, tag="ones64")
            nc.vector.memset(ones64, 1.0)

            # ~8us of dummy matmuls: HAM sees sustained PE activity and
            # releases the clock gate before fc1 arrives (K=8/8 from the
            # first real matmul instead of half-rate)
            warm_ps = psw.tile([128, 128], F32, tag="wide", name="warm_ps")
            for i in range(40):
                nc.tensor.matmul(warm_ps, ones64, ones64,
                                 start=(i == 0), stop=(i == 39))

            # ---------- sync queue: trunk inputs then the Wru stream ----------
            # x first, then w1 in 4 chunk-groups so fc1 can start early (the
            # aggregate DMA rate is the bottleneck, so queue placement of
            # these bytes doesn't matter — measured)
            xsb = cst.tile([128, KCH * B], BF16, tag="xsb")
            nc.sync.dma_start(xsb, xt)
            W1G = [(0, 7), (7, 7), (14, 6), (20, 6)]
            w1g = []
            for gi, (base, L) in enumerate(W1G):
                t = cst.tile([128, L * H], BF16, tag=f"w1g{gi}", name=f"w1g{gi}")
                nc.sync.dma_start(t, w1[:, base * H:(base + L) * H])
                w1g.append(t)

            # Wru stream + head pre-sum: pairwise TT-add tree on DVE (bf16
            # unit-stride TT hits the 2x_1P mode; tensor_reduce does not).
            # Supertile = 16 head-blocks of 500; tree halves 4 times.
            acc = [sb.tile([128, RPC], BF16, tag=f"acc{k}", name=f"acc{k}")
                   for k in range(4)]

            def tree4(wt, kc, g, h):
                # sum 4 head-blocks [128, 4*RPC] -> [128, RPC] into acc[kc]
                t3 = partp.tile([128, 2 * RPC], BF16, tag="t3",
                                name=f"t3_{kc}_{g}_{h}")
                nc.vector.tensor_add(t3, wt[:, 0:2 * RPC], wt[:, 2 * RPC:4 * RPC])
                if g == 0 and h == 0:
                    nc.vector.tensor_add(acc[kc], t3[:, 0:RPC], t3[:, RPC:2 * RPC])
                else:
                    part = partp.tile([128, RPC], BF16, tag="part",
                                      name=f"part{kc}_{g}_{h}")
                    nc.vector.tensor_add(part, t3[:, 0:RPC], t3[:, RPC:2 * RPC])
                    nc.vector.tensor_add(acc[kc], acc[kc], part)

            def tree8(wt, kc, g, h):
                # sum 8 head-blocks [128, 8*RPC] -> [128, RPC] into acc[kc]
                t2 = partp.tile([128, 4 * RPC], BF16, tag="t2",
                                name=f"t2_{kc}_{g}_{h}")
                nc.vector.tensor_add(t2, wt[:, 0:4 * RPC], wt[:, 4 * RPC:8 * RPC])
                tree4(t2, kc, g, h)

            for kc in range(4):
                for g in range(NGRP):
                    last = (kc == 3 and g == NGRP - 1)
                    if not last:
                        wt = wrupool.tile([128, GRP * RPC], BF16, tag="wru",
                                          name=f"wru_t{kc}_{g}")
                        nc.sync.dma_start(wt, wrup[kc, g])
                        t1 = partp.tile([128, 8 * RPC], BF16, tag="t1",
                                        name=f"t1_{kc}_{g}")
                        nc.vector.tensor_add(t1, wt[:, 0:8 * RPC],
                                             wt[:, 8 * RPC:16 * RPC])
                        tree8(t1, kc, g, 0)
                    else:
                        # split the final supertile into quarters so the
                        # post-stream DVE lag is one quarter-tree; quarters
                        # share one regular pool slot (no extra SBUF)
                        wt = wrupool.tile([128, GRP * RPC], BF16, tag="wru",
                                          name=f"wru_t{kc}_{g}")
                        for h in range(4):
                            nc.sync.dma_start(
                                wt[:, h * 4 * RPC:(h + 1) * 4 * RPC],
                                wrup[kc, g][:, h * 4 * RPC:(h + 1) * 4 * RPC])
                            tree4(wt[:, h * 4 * RPC:(h + 1) * 4 * RPC],
                                  kc, g, h)

            # ---------- scalar queue: small loads ----------
            b1sb = cst.tile([128, 4], F32, tag="b1sb")
            nc.scalar.dma_start(b1sb, b1p)
            b2sb = cst.tile([128, 4], F32, tag="b2sb")
            nc.scalar.dma_start(b2sb, b2p)
            w2sb = cst.tile([128, 4 * H], BF16, tag="w2sb")
            nc.scalar.dma_start(w2sb, w2p)
            wacsb = cst.tile([128, 4 * RPC], BF16, tag="wacsb")
            nc.scalar.dma_start(wacsb, wacp)
            bacsb = cst.tile([1, RPC], BF16, tag="bacsb")
            nc.scalar.dma_start(bacsb, bacp)
            augsb = cst.tile([128, 4 * AUG], BF16, tag="augsb")
            nc.scalar.dma_start(augsb, augp)
            augbsb = cst.tile([1, AUG], BF16, tag="augbsb")
            nc.scalar.dma_start(augbsb, augb)
            brusb = cst.tile([64, RPC], BF16, tag="brusb")
            nc.scalar.dma_start(brusb, brup)

            # ---------- fc1: h1T[ht] = sum_k W1[k, ht]·xT[k, b] ----------
            # kc-outer so matmuls start as soon as x + the first w1 group land
            ps1 = [psfc.tile([128, B], F32, tag="fc", name=f"ps1_{ht}")
                   for ht in range(4)]
            for gi, (base, L) in enumerate(W1G):
                for j in range(L):
                    kc = base + j
                    for ht in range(4):
                        nc.tensor.matmul(
                            ps1[ht],
                            w1g[gi][:, j * H + ht * 128:j * H + (ht + 1) * 128],
                            xsb[:, ts(kc, B)],
                            start=(kc == 0), stop=(kc == KCH - 1))
            h1loc = sb.tile([128, 4, B], BF16, tag="h1loc")
            for ht in range(4):
                nc.scalar.copy(h1loc[:, ht, :], ps1[ht])

            # ---------- fc1 cross-core reduction: ONE AllReduce (bf16).
            # Measured: AllReduce rides the CCE datapath and does NOT starve
            # the model DMA queues; AllToAll/ReduceScatter stall the Wru
            # stream for ~25-30us. AR also gives every core the full sum, so
            # no AllGather is needed at all.
            ar_din = dram.tile([128, 4, B], BF16, tag="ar_din")
            ar_dout = dram.tile([128, 4, B], BF16, tag="ar_dout",
                                addr_space="Shared")
            nc.scalar.dma_start(ar_din, h1loc)
            nc.gpsimd.collective_compute(
                "AllReduce", ADD,
                replica_groups=[list(range(NC))],
                ins=[ar_din.opt()], outs=[ar_dout.opt()],
            )
            h1r = sb.tile([128, 4, B], BF16, tag="h1r")
            nc.scalar.dma_start(h1r, ar_dout)
            h1T = []
            for ht in range(4):
                t = sb.tile([128, B], BF16, tag=f"h1T{ht}", name=f"h1T{ht}")
                nc.scalar.activation(t, h1r[:, ht, :], RELU,
                                     bias=b1sb[:, ht:ht + 1])
                h1T.append(t)

            # ---------- fc2 (redundant on every core, trivial) ----------
            hT = []
            for ht in range(4):
                p = psfc.tile([128, B], F32, tag="fc", name=f"ps2_{ht}")
                for kc in range(4):
                    nc.tensor.matmul(p, w2sb[:, kc * H + ht * 128:kc * H + (ht + 1) * 128],
                                     h1T[kc],
                                     start=(kc == 0), stop=(kc == 3))
                t = sb.tile([128, B], BF16, tag=f"hT{ht}", name=f"hT{ht}")
                nc.scalar.activation(t, p, RELU, bias=b2sb[:, ht:ht + 1])
                hT.append(t)

            # ---------- assoc heads + value_c (augmented matmul) ----------
            # row-sums for both the cache head (cols 0,1) and the rec head
            # (cols 2,3) gather in one tile -> ONE tail AllGather
            ar_in = sb.tile([128, 4], F32, tag="ar_in")
            value_sb = []
            junkA = sb.tile([128, M], F32, tag="junkA")
            for bt in range(2):
                psA = psw.tile([128, AUG], F32, tag="wide", name=f"psA{bt}")
                nc.tensor.matmul(psA, ones1, augbsb, start=True, stop=False)
                for kc in range(4):
                    nc.tensor.matmul(psA, hT[kc][:, ts(bt, 128)],
                                     augsb[:, kc * AUG:(kc + 1) * AUG],
                                     start=False, stop=(kc == 3))
                psA_sb = sb.tile([128, AUG], F32, tag=f"psAsb{bt}", name=f"psAsb{bt}")
                nc.scalar.copy(psA_sb, psA)
                advs = psA_sb[:, 0:HPC * (M + 1)].rearrange("p (n u) -> p n u", u=M + 1)
                negm = sb.tile([128, HPC], F32, tag=f"negmA{bt}", name=f"negmA{bt}")
                for n in range(HPC):
                    nc.scalar.activation(junkA, advs[:, n, 0:M], COPY,
                                         scale=-1.0 / M,
                                         accum_out=negm[:, n:n + 1])
                tmp = sb.tile([128, HPC], F32, tag=f"tmpA{bt}", name=f"tmpA{bt}")
                nc.gpsimd.tensor_add(tmp, advs[:, :, M], negm)
                q = sb.tile([128, HPC * M], F32, tag=f"qA{bt}", name=f"qA{bt}")
                nc.gpsimd.tensor_tensor(
                    out=q.rearrange("p (n m) -> p n m", m=M),
                    in0=advs[:, :, 0:M],
                    in1=tmp.broadcast_to([128, HPC, M]),
                    op=ADD)
                nc.scalar.dma_start(out_assoc[ts(bt, 128), :], q)
                value_sb.append(psA_sb[:, AUG - 1:AUG])

            # ---------- cache head (R-slice) + early row-sums ----------
            adv_c_sb = []
            for bt in range(2):
                psC = psw.tile([128, RPC], F32, tag="wide", name=f"psC{bt}")
                nc.tensor.matmul(psC, ones1, bacsb, start=True, stop=False)
                for kc in range(4):
                    nc.tensor.matmul(psC, hT[kc][:, ts(bt, 128)],
                                     wacsb[:, ts(kc, RPC)],
                                     start=False, stop=(kc == 3))
                t = sb.tile([128, RPC], F32, tag=f"advc{bt}", name=f"advc{bt}")
                nc.scalar.activation(t, psC, COPY,
                                     accum_out=ar_in[:, bt:bt + 1])
                adv_c_sb.append(t)

            # ---------- S = hT.T @ acc (+ sum_n bru), interleaved by kc ------
            psS = []
            for bt in range(2):
                p = pss.tile([128, RPC], F32, tag="s", name=f"psS{bt}")
                nc.tensor.matmul(p, ones64, brusb, start=True, stop=False)
                psS.append(p)
            for kc in range(4):
                for bt in range(2):
                    nc.tensor.matmul(psS[bt], hT[kc][:, ts(bt, 128)], acc[kc],
                                     start=False, stop=(kc == 3))
            # S row-sums straight from PSUM on the (now idle) DVE
            for bt in range(2):
                nc.vector.tensor_reduce(ar_in[:, 2 + bt:3 + bt], psS[bt],
                                        axis=mybir.AxisListType.X, op=ADD)

            # single tail AllGather of all four row-sum columns
            ag_din = dram.tile([128, 4], F32, tag="ag_din")
            ag_dout = dram.tile([NC * 128, 4], F32, tag="ag_dout",
                                addr_space="Shared")
            nc.scalar.dma_start(ag_din, ar_in)
            nc.gpsimd.collective_compute(
                "AllGather", mybir.AluOpType.bypass,
                replica_groups=[list(range(NC))],
                ins=[ag_din.opt()], outs=[ag_dout.opt()],
            )
            rall = sb.tile([128, NC * 4], F32, tag="rall")
            nc.scalar.dma_start(rall, ag_dout.rearrange("(g p) c -> p g c", p=128))
            rview = bass.AP(rall.tensor, rall.offset,
                            [rall.ap[0], [1, 4], [4, NC]])
            gsum = sb.tile([128, 4], F32, tag="gsum")
            nc.vector.tensor_reduce(gsum, rview, axis=mybir.AxisListType.X, op=ADD)
            negm = sb.tile([128, 4], F32, tag="negm")
            nc.scalar.activation(negm, gsum, COPY, scale=-1.0 / R)
            # qr on DVE straight from PSUM, qc on ACT — in parallel; each
            # output goes out as ONE DMA (both batch halves) to halve the
            # dispatch + receipt chains, split across the two HWDGE queues
            qrb = sb.tile([128, 2 * RPC], BF16, tag="qrb")
            qcb = sb.tile([128, 2 * RPC], BF16, tag="qcb")
            for bt in range(2):
                nc.vector.tensor_scalar(out=qrb[:, bt * RPC:(bt + 1) * RPC],
                                        in0=psS[bt],
                                        scalar1=negm[:, 2 + bt:3 + bt],
                                        scalar2=None, op0=ADD)
                vm = sb.tile([128, 1], F32, tag=f"vm{bt}", name=f"vm{bt}")
                nc.gpsimd.tensor_add(vm, value_sb[bt], negm[:, bt:bt + 1])
                nc.scalar.activation(qcb[:, bt * RPC:(bt + 1) * RPC],
                                     adv_c_sb[bt], IDENT, bias=vm, scale=1.0)
            nc.sync.dma_start(out_rec.rearrange("(t p) r -> p t r", p=128),
                              qrb.rearrange("p (t r) -> p t r", r=RPC))
            nc.scalar.dma_start(out_cache.rearrange("(t p) r -> p t r", p=128),
                                qcb.rearrange("p (t r) -> p t r", r=RPC))

    nc.compile()
    return nc


_CACHED = None


def _get_program():
    global _CACHED
    if _CACHED is None:
        _CACHED = build_program()
    return _CACHED


def make_in_maps(x, W1, b1, W2, b2, Wvc, bvc, Wac, bac, Wvu, bvu, Wau, bau, Wru, bru):
    f = np.float32
    x_bf = np.asarray(x, f).astype(BF)                    # [B, STATE]
    W1_bf = np.asarray(W1, f).astype(BF)                  # [STATE, H]
    W2_bf = np.asarray(W2, f).astype(BF)
    Wac_bf = np.asarray(Wac, f).astype(BF)
    Wru_bf = np.asarray(Wru, f).astype(BF)                # [64, 512, 4000]
    Wau_f = np.asarray(Wau, f)
    Wvu_f = np.asarray(Wvu, f)
    Wvc_f = np.asarray(Wvc, f).reshape(H)
    bau_f = np.asarray(bau, f)
    bvu_f = np.asarray(bvu, f)
    bvc_f = np.asarray(bvc, f).reshape(1)
    bru_bf = np.asarray(bru, f).astype(BF)
    bac_f = np.asarray(bac, f)
    b1_f = np.asarray(b1, f)
    b2_f = np.asarray(b2, f)

    # w2: [h1, h2] -> [p, kc*H + h2]
    w2p = np.ascontiguousarray(
        W2_bf.reshape(4, 128, H).transpose(1, 0, 2)).reshape(128, 4 * H)
    b1p = np.ascontiguousarray(b1_f.reshape(4, 128).T)
    b2p = np.ascontiguousarray(b2_f.reshape(4, 128).T)

    in_maps = []
    for c in range(NC):
        k0 = c * KPC_RAW
        r0 = c * RPC
        h0 = c * HPC
        # xT: [p, kc*B + b] = x[b, k0 + kc*128 + p]
        xs = np.zeros((KPC, B), BF)
        xs[:KPC_RAW] = x_bf[:, k0:k0 + KPC_RAW].T
        xt = np.ascontiguousarray(
            xs.reshape(KCH, 128, B).transpose(1, 0, 2)).reshape(128, KCH * B)
        # w1: [p, kc*H + h] = W1[k0 + kc*128 + p, h]
        w1s = np.zeros((KPC, H), BF)
        w1s[:KPC_RAW] = W1_bf[k0:k0 + KPC_RAW]
        w1p_ = np.ascontiguousarray(
            w1s.reshape(KCH, 128, H).transpose(1, 0, 2)).reshape(128, KCH * H)
        # wac: [p, kc*RPC + r] = Wac[kc*128 + p, r0 + r]
        wacp_ = np.ascontiguousarray(
            Wac_bf[:, r0:r0 + RPC].reshape(4, 128, RPC).transpose(1, 0, 2)
        ).reshape(128, 4 * RPC)
        # wru supertiles, head-block layout (contiguous blocks for the DVE
        # TT tree): [kc, g, p, blk*RPC + r] = Wru[g*GRP+blk, kc*128+p, r0+r]
        ws = Wru_bf[:, :, r0:r0 + RPC]                    # [64, 512, 500]
        a = ws.reshape(NGRP, GRP, 4, 128, RPC)            # [g, blk, kc, p, r]
        wru_p = np.ascontiguousarray(a.transpose(2, 0, 3, 1, 4)).reshape(
            4, NGRP, 128, GRP * RPC)
        # aug weights: [k, n*(M+1)+m | n*(M+1)+M | 88]
        aug_full = np.empty((H, AUG), f)
        aug_full[:, 0:HPC * (M + 1)] = np.concatenate(
            [Wau_f[h0:h0 + HPC].transpose(1, 0, 2),            # [H, 8, 10]
             Wvu_f[h0:h0 + HPC].T[:, :, None]], axis=2         # [H, 8, 1]
        ).reshape(H, HPC * (M + 1))
        aug_full[:, AUG - 1] = Wvc_f
        augp_ = np.ascontiguousarray(
            aug_full.astype(BF).reshape(4, 128, AUG).transpose(1, 0, 2)
        ).reshape(128, 4 * AUG)
        augb_ = np.empty((1, AUG), f)
        augb_[0, 0:HPC * (M + 1)] = np.concatenate(
            [bau_f[h0:h0 + HPC], bvu_f[h0:h0 + HPC, None]], axis=1
        ).reshape(HPC * (M + 1))
        augb_[0, AUG - 1] = bvc_f[0]
        m = {
            "xt": xt,
            "w1": w1p_,
            "b1p": b1p,
            "w2p": w2p,
            "b2p": b2p,
            "wacp": wacp_,
            "bacp": np.ascontiguousarray(bac_f[None, r0:r0 + RPC]).astype(BF),
            "wrup": wru_p,
            "brup": np.ascontiguousarray(bru_bf[:, r0:r0 + RPC]),
            "augp": augp_,
            "augb": augb_.astype(BF),
        }
        in_maps.append(m)
    return in_maps


def assemble(results):
    q = np.empty((B, 2 * R + NH * M), np.float32)
    for c in range(NC):
        r0 = c * RPC
        a0 = c * HPC * M
        q[:, r0:r0 + RPC] = results[c]["out_cache"]
        q[:, R + r0:R + r0 + RPC] = results[c]["out_rec"]
        q[:, 2 * R + a0:2 * R + a0 + HPC * M] = results[c]["out_assoc"]
    return q


def run(in_maps, **kw):
    nc = _get_program()
    return bass_utils.run_bass_kernel_spmd(nc, in_maps, core_ids=list(range(NC)), **kw)


def kernel(**inputs):
    in_maps = make_in_maps(**{k: np.asarray(v) for k, v in inputs.items()})
    res = run(in_maps)
    return assemble(res.results)
